# revision 15
# baseline (speedup 1.0000x reference)
"""Trainium2 Bass kernel for multi-head attention (nn_Attention).

Problem: x[8, 32, 32, 768] -> MHA(12 heads, d=64) -> out[8, 32, 32, 768].

Sharding: pure data parallel. Batch B=8 maps 1:1 onto the 8 NeuronCores;
weights are replicated. No collectives.

Per-core algorithm (N=1024 tokens, C=768), all matmuls bf16 with fp32 PSUM
accumulation. v2 redesign around two trace findings from v1: (a) the PE's
HAM clock gate re-throttles to 1.2 GHz after any >3.4us idle gap, and the
per-head softmax-normalize chain created 12 such gaps; (b) 113us of PE time
went to fp32 transposes of x/W.

  1. All input transposes moved OFF the PE: DMA f32 row tiles, DVE-cast to
     bf16, then one dma_start_transpose (XBAR block transpose) per row tile
     builds the feature-major xT/WT/PwT layouts while the PE computes.
  2. qT/kT feature-major = WT.T @ xT;  V token-major = xT.T @ WT_v with the
     v-bias added on the PSUM->SBUF copy (bias pre-broadcast at setup via a
     K=1 fp32r ones matmul, so no per-tile seed matmuls).
  3. Scores S^T[j,i] = kT.T @ qT (K=64, both heads of a pair packed into
     the PE via tile_position); E = exp(S^T/8) via ACT (no max-subtraction:
     scores ~ N(0,1)). ACT is ~111us total and runs concurrently.
  4. PV: out^T[d,i] + denominator row = [V|1].T @ E. The PV psum is
     released immediately: numerator DVE-copied to OTn, reciprocal of the
     den row computed straight out of PSUM. The 1/den broadcast is a pair-
     packed K=1 fp32r matmul (bitcast, no staging copy) emitted one window
     later, so the PE never waits on the DVE chain.
  5. Emission is a 48-step software pipeline (6 head-pairs x 8 key tiles):
     each step issues one scores chunk, the lag-4 PV chunk of the previous
     pair, and rotating filler (v_proj / next qk tiles / normalize) to keep
     the PE stream dense and HAM warm.
  6. out = OTn.T @ PwT + proj_b, DMA out per token tile.
"""

import os
import sys

for _p in ("/opt/trn_rl_repo",):
    if _p not in sys.path:
        sys.path.insert(0, _p)

import numpy as np

import concourse.bass as bass
from concourse import bacc
import concourse.mybir as mybir
from concourse.tile import TileContext

F32 = mybir.dt.float32
F32R = mybir.dt.float32r
BF16 = mybir.dt.bfloat16
EXP = mybir.ActivationFunctionType.Exp

P = 128
C = 768            # model dim
CT = C // P        # 6 c-tiles
N = 1024           # tokens per batch element
NT = N // P        # 8 token tiles
HEADS = 12
D = 64
OQK = 2 * C        # 1536 rows of q+k features
SCALE = D ** -0.5  # 0.125


def build_nc() -> bass.Bass:
    nc = bacc.Bacc(None, target_bir_lowering=False)
    x_d = nc.declare_dram_parameter("x", [N, C], F32, isOutput=False)
    qkvw_d = nc.declare_dram_parameter("qkv_w", [3 * C, C], F32, isOutput=False)
    qkvb_d = nc.declare_dram_parameter("qkv_b", [3 * C], F32, isOutput=False)
    projw_d = nc.declare_dram_parameter("proj_w", [C, C], F32, isOutput=False)
    projb_d = nc.declare_dram_parameter("proj_b", [C], F32, isOutput=False)
    out_d = nc.declare_dram_parameter("out", [N, C], F32, isOutput=True)

    with TileContext(nc) as tc:
        with (
            tc.tile_pool(name="const", bufs=1) as cpool,
            tc.tile_pool(name="ld", bufs=4) as ldp,
            tc.tile_pool(name="cv", bufs=5) as cvp,
            tc.tile_pool(name="xTp", bufs=1) as xtp,
            tc.tile_pool(name="wTp", bufs=1) as wtp,
            tc.tile_pool(name="pwp", bufs=1) as pwp,
            tc.tile_pool(name="qk", bufs=1) as qkp,
            tc.tile_pool(name="v", bufs=1) as vp,
            tc.tile_pool(name="ot", bufs=1) as otp,
            tc.tile_pool(name="e", bufs=10) as ep,
            tc.tile_pool(name="rec", bufs=2) as recp,
            tc.tile_pool(name="outs", bufs=2) as outp,
            tc.tile_pool(name="psa", bufs=2, space="PSUM") as psa,
            tc.tile_pool(name="psb", bufs=2, space="PSUM") as psb,
        ):
            # ---------------- constants ----------------
            from concourse.masks import make_identity
            ident_f = cpool.tile([P, P], F32, tag="ident_f")
            make_identity(nc, ident_f)
            ident = cpool.tile([P, P], BF16, tag="ident")
            nc.vector.tensor_copy(ident, ident_f)
            # HAM warmup: ~4us of real (non-transpose) matmuls on zeros so
            # the PE clock gate opens before the transposes/qk prologue.
            # Transpose-mode ops do not count as PE-busy for HAM.
            zeros = cpool.tile([P, 512], BF16, tag="zeros")
            nc.gpsimd.memset(zeros, 0.0)
            ps_wu = psa.tile([P, N], F32, tag="psa", name="ps_wu")
            for wi in range(16):
                nc.tensor.matmul(
                    ps_wu[:, 0:512], ident, zeros,
                    start=(wi == 0), stop=(wi == 15),
                )
            nc.vector.tensor_copy(zeros, ps_wu[:, 0:512])

            # ---------------- persistent activations ----------------
            xT = xtp.tile([P, CT, N], BF16, tag="xT")
            WT = wtp.tile([P, CT, 3 * C], BF16, tag="WT")
            PwT = pwp.tile([P, CT, C], BF16, tag="PwT")
            qkT = qkp.tile([P, HEADS, N], BF16, tag="qkT")
            V = vp.tile([P, NT, HEADS, D + 1], BF16, tag="V")
            OTn = otp.tile([P, CT, N], BF16, tag="OTn")
            nc.gpsimd.memset(V[:, :, :, D], 1.0)

            # ---------------- helpers ----------------
            def load_tile(dram_rows):
                """DMA one [128, C] f32 row tile and cast to bf16. The
                PE transpose is emitted separately (trans_tile) so it can
                be scheduled as PE filler."""
                st = ldp.tile([P, C], F32, tag="ld")
                nc.sync.dma_start(st, dram_rows)
                bt = cvp.tile([P, C], BF16, tag="cv")
                nc.vector.tensor_copy(bt, st)
                return bt

            def trans_blocks(bt, dest_slice, c0, c1):
                """bf16 PE transpose of blocks [c0,c1) into one psum bank,
                one batched DVE copy to the feature-major dest slice."""
                nb = c1 - c0
                pst = psa.tile([P, nb * P], BF16, tag="psa", name="pst")
                for i, ct in enumerate(range(c0, c1)):
                    nc.tensor.transpose(
                        pst[:, i * P : (i + 1) * P],
                        bt[:, ct * P : (ct + 1) * P],
                        ident,
                    )
                nc.vector.tensor_copy(
                    dest_slice[:, c0:c1, :], pst.rearrange("p (a b) -> p a b", b=P)
                )

            staged = {}

            def load_x(nt):
                staged["x", nt] = load_tile(x_d[nt * P : (nt + 1) * P, :])

            def trans_x(nt):
                trans_blocks(
                    staged.pop(("x", nt)), xT[:, :, nt * P : (nt + 1) * P], 0, CT
                )

            def load_w(ot):
                staged["w", ot] = load_tile(qkvw_d[ot * P : (ot + 1) * P, :])

            def trans_w(ot, half=None):
                dest = WT[:, :, ot * P : (ot + 1) * P]
                if half is None:
                    trans_blocks(staged.pop(("w", ot)), dest, 0, CT)
                elif half == 0:
                    trans_blocks(staged[("w", ot)], dest, 0, 3)
                else:
                    trans_blocks(staged.pop(("w", ot)), dest, 3, CT)

            def load_pw(ct):
                staged["pw", ct] = load_tile(projw_d[ct * P : (ct + 1) * P, :])

            def dmat_w(ot):
                nc.sync.dma_start_transpose(
                    WT[:, :, ot * P : (ot + 1) * P], staged.pop(("w", ot))
                )

            def dmat_pw(ct):
                nc.sync.dma_start_transpose(
                    PwT[:, :, ct * P : (ct + 1) * P], staged.pop(("pw", ct))
                )

            def qk_tile(ot):
                """Feature-major q/k projection for one 128-feature tile."""
                ps = psa.tile([P, N], F32, tag="psa", name="ps_qk")
                for ct in range(CT):
                    for ic in range(2):
                        nc.tensor.matmul(
                            ps[:, ic * 512 : (ic + 1) * 512],
                            WT[:, ct, ot * P : (ot + 1) * P],
                            xT[:, ct, ic * 512 : (ic + 1) * 512],
                            start=(ct == 0),
                            stop=(ct == CT - 1),
                        )
                nc.vector.tensor_scalar_add(qkT[:, ot, :], ps, bqk[:, ot : ot + 1])

            def v_proj(nt):
                """Token-major V projection for one token tile."""
                ps = psa.tile([P, N], F32, tag="psa", name="ps_v")
                for ct in range(CT):
                    for o0, ow in ((0, 512), (512, 256)):
                        nc.tensor.matmul(
                            ps[:, o0 : o0 + ow],
                            xT[:, ct, nt * P : (nt + 1) * P],
                            WT[:, ct, OQK + o0 : OQK + o0 + ow],
                            start=(ct == 0),
                            stop=(ct == CT - 1),
                        )
                nc.vector.tensor_add(
                    V[:, nt, :, 0:D],
                    ps[:, :C].rearrange("p (h d) -> p h d", d=D),
                    bv_bc.rearrange("p (h d) -> p h d", d=D),
                )

            Et = {}  # (pair, half, jt//2) -> E tile [P, 2, N]

            def scores_chunk(pair, jt):
                """S^T and exp for both heads of a pair, one key tile."""
                for half in (0, 1):
                    lo = half * D
                    ps = psa.tile([P, N], F32, tag="psa", name="ps_s")
                    for ic in range(2):
                        nc.tensor.matmul(
                            ps[:, ic * 512 : (ic + 1) * 512],
                            qkT[lo : lo + D, CT + pair, jt * P : (jt + 1) * P],
                            qkT[lo : lo + D, pair, ic * 512 : (ic + 1) * 512],
                            start=True,
                            stop=True,
                            tile_position=(lo, 0),
                        )
                    nc.scalar.activation(
                        Et[(pair, half, jt // 2)][:, jt % 2, :], ps, EXP, scale=SCALE
                    )

            def pv_chunk(h, jt, pspv):
                for ic in range(2):
                    nc.tensor.matmul(
                        pspv[0 : D + 1, ic * 512 : (ic + 1) * 512],
                        V[:, jt, h, :],
                        Et[(h // 2, h % 2, jt // 2)][
                            :, jt % 2, ic * 512 : (ic + 1) * 512
                        ],
                        start=(jt == 0),
                        stop=(jt == NT - 1),
                    )

            def pv_finish(h, pspv, den, rec, rec_r):
                """Drain the PV psum: recip chain first (so the broadcast
                matmul unblocks asap; DVE recip must read SBUF, not PSUM),
                numerator copy last."""
                nc.vector.tensor_copy(den, pspv[D : D + 1, :])
                nc.vector.reciprocal_approx_fast(rec, den)
                nc.vector.tensor_copy(rec_r, rec)
                nc.vector.tensor_copy(
                    OTn[(h % 2) * D : (h % 2) * D + D, h // 2, :], pspv[0:D, :]
                )

            def norm_pair(pair, recA, recB):
                """Broadcast 1/den of both heads (pair-packed fp32r K=1
                matmuls) and scale the numerators in place."""
                psbc = psa.tile([P, N], F32, tag="psa", name="ps_bc")
                for ic in range(2):
                    s = slice(ic * 512, (ic + 1) * 512)
                    nc.tensor.matmul(
                        psbc[:, s], onesA, recA[:, s],
                        start=True, stop=False,
                    )
                    nc.tensor.matmul(
                        psbc[:, s], onesB, recB[:, s],
                        start=False, stop=True,
                    )
                nc.vector.tensor_mul(OTn[:, pair, :], OTn[:, pair, :], psbc)

            def proj_tile(it):
                ps = psa.tile([P, N], F32, tag="psa", name="ps_o")
                for ct in range(CT):
                    for o0, ow in ((0, 512), (512, 256)):
                        nc.tensor.matmul(
                            ps[:, o0 : o0 + ow],
                            OTn[:, ct, it * P : (it + 1) * P],
                            PwT[:, ct, o0 : o0 + ow],
                            start=(ct == 0),
                            stop=(ct == CT - 1),
                        )
                outt = outp.tile([P, C], F32, tag="out")
                nc.vector.tensor_add(outt, ps[:, 0:C], pb_bc)
                nc.sync.dma_start(out_d[it * P : (it + 1) * P, :], outt)

            # ---------------- emission: 48-step pipeline ----------------
            # step s = pair*8 + jt:
            #   scores_chunk(pair, jt); pv chunks for step s-5; per-jt filler
            #   (weight transpose halves / next qk tiles / v_proj / norms)
            # chosen so every inter-chunk PE stretch exceeds the ~2.4us the
            # ACT engine needs to drain both exp chunks (keeps HAM warm).
            load_x(0)
            load_x(1)

            ones_st = cpool.tile([1, P], F32, tag="ones_st")
            nc.gpsimd.memset(ones_st, 1.0)
            ones_r = cpool.tile([1, P], F32R, tag="ones_r")
            nc.vector.tensor_copy(ones_r, ones_st)
            onesA_st = cpool.tile([1, P], F32, tag="onesA_st")
            nc.gpsimd.memset(onesA_st, 0.0)
            nc.gpsimd.memset(onesA_st[0:1, 0:D], 1.0)
            onesB_st = cpool.tile([1, P], F32, tag="onesB_st")
            nc.gpsimd.memset(onesB_st, 0.0)
            nc.gpsimd.memset(onesB_st[0:1, D:P], 1.0)
            onesA = cpool.tile([1, P], F32R, tag="onesA")
            nc.vector.tensor_copy(onesA, onesA_st)
            onesB = cpool.tile([1, P], F32R, tag="onesB")
            nc.vector.tensor_copy(onesB, onesB_st)

            for nt in range(2, NT):
                load_x(nt)
                trans_x(nt - 2)

            # bias DMAs after the x stream (small/strided; keep off Q1 head)
            bqk = cpool.tile([P, HEADS], F32, tag="bqk")
            nc.sync.dma_start(bqk, qkvb_d[0:OQK].rearrange("(t p) -> p t", p=P))
            bv_st = cpool.tile([1, C], F32, tag="bv_st")
            nc.sync.dma_start(bv_st, qkvb_d[None, OQK : 3 * C])
            pb_st = cpool.tile([1, C], F32, tag="pb_st")
            nc.sync.dma_start(pb_st, projb_d[None, :])

            load_w(0)
            trans_x(NT - 2)
            load_w(6)
            trans_x(NT - 1)
            trans_w(0)
            for ot in (1, 7, 12, 13, 14, 15, 16, 17, 2, 8):
                load_w(ot)
            trans_w(6)

            # v / proj bias broadcast (K=1 fp32r ones matmul, setup only)
            bv_r = cpool.tile([1, C], F32R, tag="bv_r")
            nc.vector.tensor_copy(bv_r, bv_st)
            pb_r = cpool.tile([1, C], F32R, tag="pb_r")
            nc.vector.tensor_copy(pb_r, pb_st)
            bv_bc = cpool.tile([P, C], BF16, tag="bv_bc")
            pb_bc = cpool.tile([P, C], BF16, tag="pb_bc")
            for src, dst in ((bv_r, bv_bc), (pb_r, pb_bc)):
                psx = psa.tile([P, N], F32, tag="psa", name="ps_bias")
                for o0, ow in ((0, 512), (512, 256)):
                    nc.tensor.matmul(
                        psx[:, o0 : o0 + ow],
                        ones_r,
                        src[:, o0 : o0 + ow],
                        start=True,
                        stop=True,
                    )
                nc.vector.tensor_copy(dst, psx[:, 0:C])

            qk_tile(0)
            trans_w(1)
            qk_tile(6)
            trans_w(7)
            dmat_w(2)
            dmat_w(8)

            pspv = {}          # head -> psum tile
            recs = {}          # head -> reciprocal row (f32r)

            LAG = 5

            def do_pv_step(s):
                q, jtp = (s - LAG) // 8, (s - LAG) % 8
                if jtp == 0:
                    pspv[2 * q] = psb.tile([P, N], F32, tag="psb", name="ps_pv")
                    pspv[2 * q + 1] = psb.tile([P, N], F32, tag="psb", name="ps_pv")
                pv_chunk(2 * q, jtp, pspv[2 * q])
                pv_chunk(2 * q + 1, jtp, pspv[2 * q + 1])
                if jtp == NT - 1:
                    for h in (2 * q, 2 * q + 1):
                        den = recp.tile([1, N], F32, tag="den", name=f"den_{h}")
                        rec = recp.tile([1, N], F32, tag="rec", name=f"rec_{h}")
                        recs[h] = recp.tile(
                            [1, N], F32R, tag="rec_r", name=f"rec_r_{h}"
                        )
                        pv_finish(h, pspv[h], den, rec, recs[h])

            for s in range(48):
                pair, jt = s // 8, s % 8
                if jt % 2 == 0:
                    for half in (0, 1):
                        Et[(pair, half, jt // 2)] = ep.tile(
                            [P, 2, N], BF16, tag="E", name=f"E_{pair}_{half}_{jt // 2}"
                        )
                scores_chunk(pair, jt)
                if s >= LAG:
                    do_pv_step(s)
                # window 0: v-block weight transposes on the PE (needed
                # within 3 steps; HAM is still cold here anyway)
                if pair == 0 and jt <= 2:
                    trans_w(12 + 2 * jt)
                    trans_w(13 + 2 * jt)
                if pair == 0 and jt >= 3:
                    v_proj(jt - 3)
                if pair == 1 and jt <= 2:
                    v_proj(jt + 5)
                # mid-kernel weight transposes ride the DMA XBAR (>=1-window
                # lead), keeping the PE stream free of transpose-mode ops
                # that HAM counts as idle
                if pair <= 2 and jt == 4:
                    dmat_w(pair + 3)
                if pair <= 2 and jt == 7:
                    dmat_w(pair + 9)
                if pair == 4 and jt <= 5:
                    dmat_pw(jt)
                # next pair q/k projections
                if pair <= 4 and jt == 2:
                    qk_tile(pair + 1)
                if pair <= 4 and jt == 5:
                    qk_tile(pair + 7)
                # stage upcoming weight tiles
                if pair <= 2 and jt == 2:
                    load_w(pair + 3)
                if pair <= 2 and jt == 5:
                    load_w(pair + 9)
                if pair == 3 and 1 <= jt <= 6:
                    load_pw(jt - 1)
                # normalize previous pair
                if pair >= 1 and jt == 7:
                    norm_pair(pair - 1, recs[2 * (pair - 1)], recs[2 * (pair - 1) + 1])

            for s in range(48, 48 + LAG):
                do_pv_step(s)
            norm_pair(5, recs[10], recs[11])

            for it in range(NT):
                proj_tile(it)

    nc.compile()
    return nc


_NC_CACHE = None


def _get_nc():
    global _NC_CACHE
    if _NC_CACHE is None:
        _NC_CACHE = build_nc()
    return _NC_CACHE


def run(inputs, trace=False, tmpdir=None):
    """Run on 8 NeuronCores; returns (out[8,32,32,768], BassKernelResults)."""
    from concourse.bass_utils import run_bass_kernel_spmd

    x = np.asarray(inputs["x"], dtype=np.float32)
    B, H, W, Cc = x.shape
    xf = np.ascontiguousarray(x.reshape(B, H * W, Cc))
    qkv_w = np.ascontiguousarray(np.asarray(inputs["qkv_w"], dtype=np.float32))
    qkv_b = np.ascontiguousarray(np.asarray(inputs["qkv_b"], dtype=np.float32))
    proj_w = np.ascontiguousarray(np.asarray(inputs["proj_w"], dtype=np.float32))
    proj_b = np.ascontiguousarray(np.asarray(inputs["proj_b"], dtype=np.float32))

    nc = _get_nc()
    in_maps = [
        {
            "x": xf[b],
            "qkv_w": qkv_w,
            "qkv_b": qkv_b,
            "proj_w": proj_w,
            "proj_b": proj_b,
        }
        for b in range(B)
    ]
    res = run_bass_kernel_spmd(nc, in_maps, list(range(B)), trace=trace, tmpdir=tmpdir)
    out = np.stack([res.results[b]["out"] for b in range(B)])
    return out.reshape(B, H, W, Cc).astype(np.float32), res


def kernel(x, qkv_w, qkv_b, proj_w, proj_b):
    out, _ = run(
        {
            "x": x,
            "qkv_w": qkv_w,
            "qkv_b": qkv_b,
            "proj_w": proj_w,
            "proj_b": proj_b,
        }
    )
    return out


# revision 17
# speedup vs baseline: 1.1097x; 1.1097x over previous
"""Trainium2 Bass kernel for multi-head attention (nn_Attention).

Problem: x[8, 32, 32, 768] -> MHA(12 heads, d=64) -> out[8, 32, 32, 768].

Sharding: pure data parallel. Batch B=8 maps 1:1 onto the 8 NeuronCores;
weights are replicated. No collectives.

Per-core algorithm (N=1024 tokens, C=768), all matmuls bf16 with fp32 PSUM
accumulation. v2 redesign around two trace findings from v1: (a) the PE's
HAM clock gate re-throttles to 1.2 GHz after any >3.4us idle gap, and the
per-head softmax-normalize chain created 12 such gaps; (b) 113us of PE time
went to fp32 transposes of x/W.

  1. All input transposes moved OFF the PE: DMA f32 row tiles, DVE-cast to
     bf16, then one dma_start_transpose (XBAR block transpose) per row tile
     builds the feature-major xT/WT/PwT layouts while the PE computes.
  2. qT/kT feature-major = WT.T @ xT;  V token-major = xT.T @ WT_v with the
     v-bias added on the PSUM->SBUF copy (bias pre-broadcast at setup via a
     K=1 fp32r ones matmul, so no per-tile seed matmuls).
  3. Scores S^T[j,i] = kT.T @ qT (K=64, both heads of a pair packed into
     the PE via tile_position); E = exp(S^T/8) via ACT (no max-subtraction:
     scores ~ N(0,1)). ACT is ~111us total and runs concurrently.
  4. PV: out^T[d,i] + denominator row = [V|1].T @ E. The PV psum is
     released immediately: numerator DVE-copied to OTn, reciprocal of the
     den row computed straight out of PSUM. The 1/den broadcast is a pair-
     packed K=1 fp32r matmul (bitcast, no staging copy) emitted one window
     later, so the PE never waits on the DVE chain.
  5. Emission is a 48-step software pipeline (6 head-pairs x 8 key tiles):
     each step issues one scores chunk, the lag-4 PV chunk of the previous
     pair, and rotating filler (v_proj / next qk tiles / normalize) to keep
     the PE stream dense and HAM warm.
  6. out = OTn.T @ PwT + proj_b, DMA out per token tile.
"""

import os
import sys

for _p in ("/opt/trn_rl_repo",):
    if _p not in sys.path:
        sys.path.insert(0, _p)

import numpy as np

import concourse.bass as bass
from concourse import bacc
import concourse.mybir as mybir
from concourse.tile import TileContext

F32 = mybir.dt.float32
F32R = mybir.dt.float32r
BF16 = mybir.dt.bfloat16
EXP = mybir.ActivationFunctionType.Exp

P = 128
C = 768            # model dim
CT = C // P        # 6 c-tiles
N = 1024           # tokens per batch element
NT = N // P        # 8 token tiles
HEADS = 12
D = 64
OQK = 2 * C        # 1536 rows of q+k features
SCALE = D ** -0.5  # 0.125


def build_nc() -> bass.Bass:
    nc = bacc.Bacc(None, target_bir_lowering=False)
    x_d = nc.declare_dram_parameter("x", [N, C], F32, isOutput=False)
    qkvw_d = nc.declare_dram_parameter("qkv_w", [3 * C, C], F32, isOutput=False)
    qkvb_d = nc.declare_dram_parameter("qkv_b", [3 * C], F32, isOutput=False)
    projw_d = nc.declare_dram_parameter("proj_w", [C, C], F32, isOutput=False)
    projb_d = nc.declare_dram_parameter("proj_b", [C], F32, isOutput=False)
    out_d = nc.declare_dram_parameter("out", [N, C], F32, isOutput=True)

    with TileContext(nc) as tc:
        with (
            tc.tile_pool(name="const", bufs=1) as cpool,
            tc.tile_pool(name="ld", bufs=4) as ldp,
            tc.tile_pool(name="cv", bufs=5) as cvp,
            tc.tile_pool(name="xTp", bufs=1) as xtp,
            tc.tile_pool(name="wTp", bufs=1) as wtp,
            tc.tile_pool(name="pwp", bufs=1) as pwp,
            tc.tile_pool(name="qk", bufs=1) as qkp,
            tc.tile_pool(name="v", bufs=1) as vp,
            tc.tile_pool(name="ot", bufs=1) as otp,
            tc.tile_pool(name="e", bufs=10) as ep,
            tc.tile_pool(name="rec", bufs=2) as recp,
            tc.tile_pool(name="outs", bufs=2) as outp,
            tc.tile_pool(name="psa", bufs=2, space="PSUM") as psa,
            tc.tile_pool(name="psb", bufs=2, space="PSUM") as psb,
        ):
            # ---------------- constants ----------------
            from concourse.masks import make_identity
            ident_f = cpool.tile([P, P], F32, tag="ident_f")
            make_identity(nc, ident_f)
            ident = cpool.tile([P, P], BF16, tag="ident")
            nc.vector.tensor_copy(ident, ident_f)
            # HAM warmup: ~4us of real (non-transpose) matmuls on zeros so
            # the PE clock gate opens before the transposes/qk prologue.
            # Transpose-mode ops do not count as PE-busy for HAM.
            zeros = cpool.tile([P, 512], BF16, tag="zeros")
            nc.gpsimd.memset(zeros, 0.0)
            ps_wu = psa.tile([P, N], F32, tag="psa", name="ps_wu")
            for wi in range(16):
                nc.tensor.matmul(
                    ps_wu[:, 0:512], ident, zeros,
                    start=(wi == 0), stop=(wi == 15),
                )
            nc.vector.tensor_copy(zeros, ps_wu[:, 0:512])

            # ---------------- persistent activations ----------------
            xT = xtp.tile([P, CT, N], BF16, tag="xT")
            WT = wtp.tile([P, CT, 3 * C], BF16, tag="WT")
            PwT = pwp.tile([P, CT, C], BF16, tag="PwT")
            qkT = qkp.tile([P, HEADS, N], BF16, tag="qkT")
            V = vp.tile([P, NT, HEADS, D + 1], BF16, tag="V")
            OTn = otp.tile([P, CT, N], BF16, tag="OTn")
            nc.gpsimd.memset(V[:, :, :, D], 1.0)

            # ---------------- helpers ----------------
            def load_tile(dram_rows):
                """DMA one [128, C] f32 row tile and cast to bf16. The
                PE transpose is emitted separately (trans_tile) so it can
                be scheduled as PE filler."""
                st = ldp.tile([P, C], F32, tag="ld")
                nc.sync.dma_start(st, dram_rows)
                bt = cvp.tile([P, C], BF16, tag="cv")
                nc.vector.tensor_copy(bt, st)
                return bt

            def trans_blocks(bt, dest_slice, c0, c1):
                """bf16 PE transpose of blocks [c0,c1) into one psum bank,
                one batched DVE copy to the feature-major dest slice."""
                nb = c1 - c0
                pst = psa.tile([P, nb * P], BF16, tag="psa", name="pst")
                for i, ct in enumerate(range(c0, c1)):
                    nc.tensor.transpose(
                        pst[:, i * P : (i + 1) * P],
                        bt[:, ct * P : (ct + 1) * P],
                        ident,
                    )
                nc.vector.tensor_copy(
                    dest_slice[:, c0:c1, :], pst.rearrange("p (a b) -> p a b", b=P)
                )

            staged = {}

            def load_x(nt):
                staged["x", nt] = load_tile(x_d[nt * P : (nt + 1) * P, :])

            def trans_x(nt):
                trans_blocks(
                    staged.pop(("x", nt)), xT[:, :, nt * P : (nt + 1) * P], 0, CT
                )

            def load_w(ot):
                staged["w", ot] = load_tile(qkvw_d[ot * P : (ot + 1) * P, :])

            def trans_w(ot, half=None):
                dest = WT[:, :, ot * P : (ot + 1) * P]
                if half is None:
                    trans_blocks(staged.pop(("w", ot)), dest, 0, CT)
                elif half == 0:
                    trans_blocks(staged[("w", ot)], dest, 0, 3)
                else:
                    trans_blocks(staged.pop(("w", ot)), dest, 3, CT)

            def load_pw(ct):
                staged["pw", ct] = load_tile(projw_d[ct * P : (ct + 1) * P, :])

            def trans_pw(ct):
                trans_blocks(
                    staged.pop(("pw", ct)), PwT[:, :, ct * P : (ct + 1) * P], 0, CT
                )

            def qk_tile(ot):
                """Feature-major q/k projection for one 128-feature tile."""
                ps = psa.tile([P, N], F32, tag="psa", name="ps_qk")
                for ct in range(CT):
                    for ic in range(2):
                        nc.tensor.matmul(
                            ps[:, ic * 512 : (ic + 1) * 512],
                            WT[:, ct, ot * P : (ot + 1) * P],
                            xT[:, ct, ic * 512 : (ic + 1) * 512],
                            start=(ct == 0),
                            stop=(ct == CT - 1),
                        )
                nc.vector.tensor_scalar_add(qkT[:, ot, :], ps, bqk[:, ot : ot + 1])

            def v_proj(nt):
                """Token-major V projection for one token tile."""
                ps = psa.tile([P, N], F32, tag="psa", name="ps_v")
                for ct in range(CT):
                    for o0, ow in ((0, 512), (512, 256)):
                        nc.tensor.matmul(
                            ps[:, o0 : o0 + ow],
                            xT[:, ct, nt * P : (nt + 1) * P],
                            WT[:, ct, OQK + o0 : OQK + o0 + ow],
                            start=(ct == 0),
                            stop=(ct == CT - 1),
                        )
                nc.vector.tensor_add(
                    V[:, nt, :, 0:D],
                    ps[:, :C].rearrange("p (h d) -> p h d", d=D),
                    bv_bc.rearrange("p (h d) -> p h d", d=D),
                )

            Et = {}  # (pair, half, jt//2) -> E tile [P, 2, N]

            def scores_chunk(pair, jt):
                """S^T and exp for both heads of a pair, one key tile."""
                for half in (0, 1):
                    lo = half * D
                    ps = psa.tile([P, N], F32, tag="psa", name="ps_s")
                    for ic in range(2):
                        nc.tensor.matmul(
                            ps[:, ic * 512 : (ic + 1) * 512],
                            qkT[lo : lo + D, CT + pair, jt * P : (jt + 1) * P],
                            qkT[lo : lo + D, pair, ic * 512 : (ic + 1) * 512],
                            start=True,
                            stop=True,
                            tile_position=(lo, 0),
                        )
                    nc.scalar.activation(
                        Et[(pair, half, jt // 2)][:, jt % 2, :], ps, EXP, scale=SCALE
                    )

            def pv_chunk(h, jt, pspv):
                for ic in range(2):
                    nc.tensor.matmul(
                        pspv[0 : D + 1, ic * 512 : (ic + 1) * 512],
                        V[:, jt, h, :],
                        Et[(h // 2, h % 2, jt // 2)][
                            :, jt % 2, ic * 512 : (ic + 1) * 512
                        ],
                        start=(jt == 0),
                        stop=(jt == NT - 1),
                    )

            def pv_finish(h, pspv, den, rec, rec_r):
                """Drain the PV psum: recip chain first (so the broadcast
                matmul unblocks asap; DVE recip must read SBUF, not PSUM),
                numerator copy last."""
                nc.vector.tensor_copy(den, pspv[D : D + 1, :])
                nc.vector.reciprocal_approx_fast(rec, den)
                nc.vector.tensor_copy(rec_r, rec)
                nc.vector.tensor_copy(
                    OTn[(h % 2) * D : (h % 2) * D + D, h // 2, :], pspv[0:D, :]
                )

            def norm_pair(pair, recA, recB):
                """Broadcast 1/den of both heads (pair-packed fp32r K=1
                matmuls) and scale the numerators in place."""
                psbc = psa.tile([P, N], F32, tag="psa", name="ps_bc")
                for ic in range(2):
                    s = slice(ic * 512, (ic + 1) * 512)
                    nc.tensor.matmul(
                        psbc[:, s], onesA, recA[:, s],
                        start=True, stop=False,
                    )
                    nc.tensor.matmul(
                        psbc[:, s], onesB, recB[:, s],
                        start=False, stop=True,
                    )
                nc.vector.tensor_mul(OTn[:, pair, :], OTn[:, pair, :], psbc)

            def proj_tile(it):
                ps = psa.tile([P, N], F32, tag="psa", name="ps_o")
                for ct in range(CT):
                    for o0, ow in ((0, 512), (512, 256)):
                        nc.tensor.matmul(
                            ps[:, o0 : o0 + ow],
                            OTn[:, ct, it * P : (it + 1) * P],
                            PwT[:, ct, o0 : o0 + ow],
                            start=(ct == 0),
                            stop=(ct == CT - 1),
                        )
                outt = outp.tile([P, C], F32, tag="out")
                nc.vector.tensor_add(outt, ps[:, 0:C], pb_bc)
                nc.sync.dma_start(out_d[it * P : (it + 1) * P, :], outt)

            # ---------------- emission: 48-step pipeline ----------------
            # step s = pair*8 + jt:
            #   scores_chunk(pair, jt); pv chunks for step s-5; per-jt filler
            #   (weight transpose halves / next qk tiles / v_proj / norms)
            # chosen so every inter-chunk PE stretch exceeds the ~2.4us the
            # ACT engine needs to drain both exp chunks (keeps HAM warm).
            load_x(0)
            load_x(1)

            ones_st = cpool.tile([1, P], F32, tag="ones_st")
            nc.gpsimd.memset(ones_st, 1.0)
            ones_r = cpool.tile([1, P], F32R, tag="ones_r")
            nc.vector.tensor_copy(ones_r, ones_st)
            onesA_st = cpool.tile([1, P], F32, tag="onesA_st")
            nc.gpsimd.memset(onesA_st, 0.0)
            nc.gpsimd.memset(onesA_st[0:1, 0:D], 1.0)
            onesB_st = cpool.tile([1, P], F32, tag="onesB_st")
            nc.gpsimd.memset(onesB_st, 0.0)
            nc.gpsimd.memset(onesB_st[0:1, D:P], 1.0)
            onesA = cpool.tile([1, P], F32R, tag="onesA")
            nc.vector.tensor_copy(onesA, onesA_st)
            onesB = cpool.tile([1, P], F32R, tag="onesB")
            nc.vector.tensor_copy(onesB, onesB_st)

            for nt in range(2, NT):
                load_x(nt)
                trans_x(nt - 2)

            # bias DMAs after the x stream (small/strided; keep off Q1 head)
            bqk = cpool.tile([P, HEADS], F32, tag="bqk")
            nc.sync.dma_start(bqk, qkvb_d[0:OQK].rearrange("(t p) -> p t", p=P))
            bv_st = cpool.tile([1, C], F32, tag="bv_st")
            nc.sync.dma_start(bv_st, qkvb_d[None, OQK : 3 * C])
            pb_st = cpool.tile([1, C], F32, tag="pb_st")
            nc.sync.dma_start(pb_st, projb_d[None, :])

            load_w(0)
            trans_x(NT - 2)
            load_w(6)
            trans_x(NT - 1)
            trans_w(0)
            for ot in (1, 7, 12, 13, 14, 15, 16, 17, 2, 8):
                load_w(ot)
            trans_w(6)

            # v / proj bias broadcast (K=1 fp32r ones matmul, setup only)
            bv_r = cpool.tile([1, C], F32R, tag="bv_r")
            nc.vector.tensor_copy(bv_r, bv_st)
            pb_r = cpool.tile([1, C], F32R, tag="pb_r")
            nc.vector.tensor_copy(pb_r, pb_st)
            bv_bc = cpool.tile([P, C], BF16, tag="bv_bc")
            pb_bc = cpool.tile([P, C], BF16, tag="pb_bc")
            for src, dst in ((bv_r, bv_bc), (pb_r, pb_bc)):
                psx = psa.tile([P, N], F32, tag="psa", name="ps_bias")
                for o0, ow in ((0, 512), (512, 256)):
                    nc.tensor.matmul(
                        psx[:, o0 : o0 + ow],
                        ones_r,
                        src[:, o0 : o0 + ow],
                        start=True,
                        stop=True,
                    )
                nc.vector.tensor_copy(dst, psx[:, 0:C])

            qk_tile(0)
            trans_w(1)
            qk_tile(6)
            trans_w(7)

            pspv = {}          # head -> psum tile
            recs = {}          # head -> reciprocal row (f32r)

            LAG = 5

            def do_pv_step(s):
                q, jtp = (s - LAG) // 8, (s - LAG) % 8
                if jtp == 0:
                    pspv[2 * q] = psb.tile([P, N], F32, tag="psb", name="ps_pv")
                    pspv[2 * q + 1] = psb.tile([P, N], F32, tag="psb", name="ps_pv")
                pv_chunk(2 * q, jtp, pspv[2 * q])
                pv_chunk(2 * q + 1, jtp, pspv[2 * q + 1])
                if jtp == NT - 1:
                    for h in (2 * q, 2 * q + 1):
                        den = recp.tile([1, N], F32, tag="den", name=f"den_{h}")
                        rec = recp.tile([1, N], F32, tag="rec", name=f"rec_{h}")
                        recs[h] = recp.tile(
                            [1, N], F32R, tag="rec_r", name=f"rec_r_{h}"
                        )
                        pv_finish(h, pspv[h], den, rec, recs[h])

            for s in range(48):
                pair, jt = s // 8, s % 8
                if jt % 2 == 0:
                    for half in (0, 1):
                        Et[(pair, half, jt // 2)] = ep.tile(
                            [P, 2, N], BF16, tag="E", name=f"E_{pair}_{half}_{jt // 2}"
                        )
                scores_chunk(pair, jt)
                if s >= LAG:
                    do_pv_step(s)
                # window 0: v-block weight transposes first (pop order must
                # match load order for the cv staging ring), then W2/W8
                if pair == 0:
                    if jt <= 2:
                        trans_w(12 + 2 * jt)
                        trans_w(13 + 2 * jt)
                    elif jt == 3:
                        trans_w(2, half=0)
                    elif jt == 4:
                        trans_w(2, half=1)
                    elif jt == 6:
                        trans_w(8, half=0)
                    elif jt == 7:
                        trans_w(8, half=1)
                if pair == 0 and jt >= 3:
                    v_proj(jt - 3)
                if pair == 1 and jt <= 2:
                    v_proj(jt + 5)
                # weight-transpose halves as spread filler
                if 1 <= pair <= 3:
                    if jt == 0:
                        trans_w(pair + 2, half=0)
                    elif jt == 1:
                        trans_w(pair + 2, half=1)
                    elif jt == 3:
                        trans_w(pair + 8, half=0)
                    elif jt == 4:
                        trans_w(pair + 8, half=1)
                if pair == 4 and jt in (0, 1, 3):
                    trans_pw((0, 1, None, 2)[jt])
                if pair == 5 and jt in (0, 1, 2):
                    trans_pw(jt + 3)
                # next pair q/k projections
                if pair <= 4 and jt == 2:
                    qk_tile(pair + 1)
                if pair <= 4 and jt == 5:
                    qk_tile(pair + 7)
                # stage upcoming weight tiles
                if pair <= 2 and jt == 2:
                    load_w(pair + 3)
                if pair <= 2 and jt == 5:
                    load_w(pair + 9)
                if pair == 3 and 1 <= jt <= 6:
                    load_pw(jt - 1)
                # normalize previous pair
                if pair >= 1 and jt == 7:
                    norm_pair(pair - 1, recs[2 * (pair - 1)], recs[2 * (pair - 1) + 1])

            for s in range(48, 48 + LAG):
                do_pv_step(s)
            norm_pair(5, recs[10], recs[11])

            for it in range(NT):
                proj_tile(it)

    nc.compile()
    return nc


_NC_CACHE = None


def _get_nc():
    global _NC_CACHE
    if _NC_CACHE is None:
        _NC_CACHE = build_nc()
    return _NC_CACHE


def run(inputs, trace=False, tmpdir=None):
    """Run on 8 NeuronCores; returns (out[8,32,32,768], BassKernelResults)."""
    from concourse.bass_utils import run_bass_kernel_spmd

    x = np.asarray(inputs["x"], dtype=np.float32)
    B, H, W, Cc = x.shape
    xf = np.ascontiguousarray(x.reshape(B, H * W, Cc))
    qkv_w = np.ascontiguousarray(np.asarray(inputs["qkv_w"], dtype=np.float32))
    qkv_b = np.ascontiguousarray(np.asarray(inputs["qkv_b"], dtype=np.float32))
    proj_w = np.ascontiguousarray(np.asarray(inputs["proj_w"], dtype=np.float32))
    proj_b = np.ascontiguousarray(np.asarray(inputs["proj_b"], dtype=np.float32))

    nc = _get_nc()
    in_maps = [
        {
            "x": xf[b],
            "qkv_w": qkv_w,
            "qkv_b": qkv_b,
            "proj_w": proj_w,
            "proj_b": proj_b,
        }
        for b in range(B)
    ]
    res = run_bass_kernel_spmd(nc, in_maps, list(range(B)), trace=trace, tmpdir=tmpdir)
    out = np.stack([res.results[b]["out"] for b in range(B)])
    return out.reshape(B, H, W, Cc).astype(np.float32), res


def kernel(x, qkv_w, qkv_b, proj_w, proj_b):
    out, _ = run(
        {
            "x": x,
            "qkv_w": qkv_w,
            "qkv_b": qkv_b,
            "proj_w": proj_w,
            "proj_b": proj_b,
        }
    )
    return out
